# revision 28
# baseline (speedup 1.0000x reference)
"""Document-mask attention (B=1, H=16, N=4096, D=64) on 8 trn2 NeuronCores.

Strategy
--------
Head-sharded: core c computes heads (2c, 2c+1) over the full sequence.
The document mask is block-diagonal with contiguous blocks (document_id is
sorted), so per document d with token range [s, e) the attention is an
independent dense softmax(Q_d K_d^T / 8) V_d.  We only compute
within-document score blocks (~1/13 of the dense FLOPs).

Per (doc, head) on device, in S^T layout (keys on partitions):
  for each 128-key chunk i:   ST_i = KT_i^T @ QT_doc      (PE, contraction D=64)
  ET = exp(ST)                                            (ACT, chunks batched)
  OT += VO_i^T @ ET_i  accumulated over chunks            (PE, contraction 128)
where VO = [V | 1] (65 columns).  Row 64 of OT is the softmax denominator;
normalization + final transpose happen on the host during the unshard step.

Matmul operands are fp16 (PSUM accumulation stays fp32): measured end-to-end
scale-relative absmax error ~8e-4 vs the fp32 reference, 4x faster on the PE
than fp32 (which lowers to 2 HW passes at 2 cycles/column).

Host-side layout prep (part of sharding): Q^T and K^T are packed 2-heads-on-
128-partitions (scale 1/8 folded into Q); K and V are padded per document to
128-multiples with zero rows (padded keys get score 0 -> exp 1, annihilated
by zero V and zero ones-column), making every PE operand a full-partition
rectangular AP and masking completely free.
"""

import math
import os
import sys

import numpy as np

sys.path.insert(0, "/opt/trn_rl_repo")
os.environ.setdefault("MYCRO_LOCAL_CACHE", "1")

B, H, N, D = 1, 16, 4096, 64
N_CORES = 8
HEADS_PER_CORE = H // N_CORES  # 2
SCALE = 1.0 / math.sqrt(D)

_prog_cache = {}


def _doc_segments(document_id):
    """[(start, end, padded_block_start, n_blocks)] from sorted doc ids."""
    doc = np.asarray(document_id)
    assert doc.shape == (N,)
    bounds = [0] + list(np.nonzero(doc[1:] != doc[:-1])[0] + 1) + [N]
    segs = []
    b0 = 0
    for s, e in zip(bounds[:-1], bounds[1:]):
        nb = -(-(e - s) // 128)
        segs.append((int(s), int(e), b0, nb))
        b0 += nb
    return segs


def _doc_groups(segs, n_groups=4, pieces=None):
    """Split docs into contiguous groups for chunked DMA."""
    if pieces is not None:
        cuts = [0] + [min(p, len(segs)) for p in pieces]
        out = [segs[a:b] for a, b in zip(cuts[:-1], cuts[1:]) if b > a]
        if cuts[-1] < len(segs):
            out.append(segs[cuts[-1]:])
        return out
    per = -(-len(segs) // n_groups)
    return [segs[i:i + per] for i in range(0, len(segs), per)]


def _build_program(segs):
    """One SPMD Bass program (same for all cores; doc structure is global)."""
    import concourse.bacc as bacc
    import concourse.bass as bass
    import concourse.tile as tile
    from concourse import mybir

    class LightTailTileContext(tile.TileContext):
        # Tile's stock tail is drain + barrier + sem-clear + barrier (~12us
        # measured).  For a single-shot NEFF the trailing barrier only
        # synchronizes engine retirement; drop it and use the cheaper
        # sem-only barrier after the sem clears.
        def _drain_and_barrier(self, tick_clock, wait_clock):
            from concourse.tile import ScopedClock
            drain_inst = self.nc.sync.drain()
            wait_clock.add_sem_waits(
                drain_inst.ins, ScopedClock({None: tick_clock.global_clock})
            )
            self.nc.all_engine_barrier()
            popped = self.nc._tile_sem_poison_stack.pop()
            assert popped is self._sem_poison
            self.nc.clear_and_free_semaphores(
                list(self.sems.allocated().values())
            )

    f32 = mybir.dt.float32
    f16 = mybir.dt.float16
    nblk = sum(nb for (_, _, _, nb) in segs)

    nc = bacc.Bacc("TRN2", target_bir_lowering=False, debug=False,
                   num_devices=N_CORES)

    qt_d = nc.dram_tensor("qt", [128, N], f16, kind="ExternalInput")
    kt_d = nc.dram_tensor("kt", [128, nblk * 128], f16, kind="ExternalInput")
    vo_d = [nc.dram_tensor(f"vo{h}", [128, nblk * 65], f16, kind="ExternalInput")
            for h in range(HEADS_PER_CORE)]
    ot_d = nc.dram_tensor("ot", [65, HEADS_PER_CORE, N], f16,
                          kind="ExternalOutput")

    et_bufs = 10

    with LightTailTileContext(nc, pool_alloc_mode="queue") as tc:
        with (
            tc.tile_pool(name="big", bufs=1) as big,
            tc.tile_pool(name="et_pool", bufs=et_bufs) as et_pool,
            tc.tile_pool(name="st_pool", bufs=2, space=bass.MemorySpace.PSUM) as st_pool,
            tc.tile_pool(name="ot_pool", bufs=2, space=bass.MemorySpace.PSUM) as ot_pool,
        ):
            qt_t = big.tile([128, N], f16, name="qt_t")
            kt_t = big.tile([128, nblk * 128], f16, name="kt_t")
            vo_t = [big.tile([128, nblk * 65], f16, name=f"vo_t{h}")
                    for h in range(HEADS_PER_CORE)]
            ot_sb = big.tile([65, HEADS_PER_CORE, N], f16, name="ot_sb")

            # Input DMAs.  ~0.7us descriptor-gen runs on the ISSUING engine's
            # sequencer (only sync/scalar HWDGE + gpsimd SWDGE can issue),
            # and transfers are FIFO per ring, so ring order = priority.
            # Critical path to the first exp is doc-0 qt+kt: qt0 rides the
            # scalar ring alone (descgen overlaps the ACT table load), kt0
            # is first on sync; the bulk qt/kt pieces interleave behind it
            # on sync, vo on gpsimd.  Scalar then carries only exps.
            def qt_piece(eng, grp):
                gs, ge = grp[0][0], grp[-1][1]
                eng.dma_start(qt_t[:, gs:ge], qt_d[:, gs:ge])

            def kt_piece(eng, grp):
                gb0, gb1 = grp[0][2], grp[-1][2] + grp[-1][3]
                eng.dma_start(kt_t[:, 128 * gb0:128 * gb1],
                              kt_d[:, 128 * gb0:128 * gb1])

            def vo_piece(grp):
                gb0, gb1 = grp[0][2], grp[-1][2] + grp[-1][3]
                for h in range(HEADS_PER_CORE):
                    nc.gpsimd.dma_start(vo_t[h][:, 65 * gb0:65 * gb1],
                                        vo_d[h][:, 65 * gb0:65 * gb1])

            qt_grps = _doc_groups(segs, pieces=(1, 3, 6, 10, 16))
            kt_grps = _doc_groups(segs, pieces=(1, 3, 6, 10, 16))
            vo_grps = _doc_groups(segs, pieces=(2, 8, 16))
            # three parallel chains, ordered by per-doc need times.  The
            # early transfer rate is low (~50-100 GB/s/ring while all 8
            # cores ramp at once), so doc0's kt goes out block-by-block to
            # pull the first S+exp as early as possible:
            #   scalar: qt0 (alone, overlaps ACT table load)
            #   sync:   kt0a kt0b kt1 qt1 kt2 qt2 kt3 qt3 kt4 qt4
            #   gpsimd: voA voB voC      (vo trails PV by 2 steps)
            qt_piece(nc.scalar, qt_grps[0])
            b00 = segs[0][2]
            nc.sync.dma_start(kt_t[:, 128 * b00:128 * (b00 + 1)],
                              kt_d[:, 128 * b00:128 * (b00 + 1)])
            nc.sync.dma_start(kt_t[:, 128 * (b00 + 1):128 * (b00 + segs[0][3])],
                              kt_d[:, 128 * (b00 + 1):128 * (b00 + segs[0][3])])
            vo_piece(vo_grps[0])
            kt_piece(nc.sync, kt_grps[1])
            qt_piece(nc.sync, qt_grps[1])
            kt_piece(nc.sync, kt_grps[2])
            qt_piece(nc.sync, qt_grps[2])
            vo_piece(vo_grps[1])
            kt_piece(nc.sync, kt_grps[3])
            qt_piece(nc.sync, qt_grps[3])
            vo_piece(vo_grps[2])
            kt_piece(nc.sync, kt_grps[4])
            qt_piece(nc.sync, qt_grps[4])

            # (no dummy exp needed: walrus inserts the ACT table load before
            # the first EXP in the queue, which executes right at main start)

            # software-pipelined step loop; steps are <=256 queries so the
            # ST tile is <=3 PSUM banks and OT is 1 bank, letting BOTH pools
            # double-buffer within the 8 banks (3*2 + 1*2).  S matmuls for
            # step k+1 then overlap EXP(k) instead of serializing behind its
            # bank drain -- the ACT stream runs gapless.
            steps = []
            for d, (s, e, b0, nb) in enumerate(segs):
                L = e - s
                if L <= 256:
                    steps.append((s, e, b0, nb, 0, L))
                else:
                    steps.append((s, e, b0, nb, 0, L // 2))
                    steps.append((s, e, b0, nb, L // 2, L - L // 2))

            def emit_pv(et, step):
                s, e, b0, nb, g0, gl = step
                ot_ps = ot_pool.tile([65, HEADS_PER_CORE, 256], f32,
                                     tag="ot_ps", name="ot_ps")
                for h in range(HEADS_PER_CORE):
                    for i in range(nb):
                        nc.tensor.matmul(
                            ot_ps[:, h, :gl],
                            vo_t[h][:, 65 * (b0 + i):65 * (b0 + i) + 65],
                            et[:, h, i, :gl],
                            start=(i == 0), stop=(i == nb - 1),
                        )
                nc.vector.tensor_copy(
                    ot_sb[:, :, s + g0:s + g0 + gl], ot_ps[:, :, :gl]
                )

            from collections import deque
            pending = deque()
            for step_idx, step in enumerate(steps):
                s, e, b0, nb, g0, gl = step
                # h0/h1 S matmuls interleaved so adjacent pairs run
                # concurrently on PE row groups (h0 rows 0-63 / h1 64-127)
                st = st_pool.tile([128, HEADS_PER_CORE, nb, 256], f32,
                                  tag="st", name="st")
                for j in range(nb):
                    i = b0 + j
                    for h in range(HEADS_PER_CORE):
                        nc.tensor.matmul(
                            st[:, h, j, :gl],
                            kt_t[64 * h:64 * h + 64, 128 * i:128 * (i + 1)],
                            qt_t[64 * h:64 * h + 64, s + g0:s + g0 + gl],
                            start=True, stop=True,
                        )
                et = et_pool.tile([128, HEADS_PER_CORE, nb, 256], f16,
                                  tag="et", name="et")
                if step_idx == 0:
                    # ramp: per-block exps start right as kt blocks trickle
                    # in (ACT is data-starved here anyway)
                    for j in range(nb):
                        nc.scalar.activation(
                            et[:, :, j:j + 1, :gl], st[:, :, j:j + 1, :gl],
                            mybir.ActivationFunctionType.Exp,
                        )
                elif step_idx == 1:
                    for h in range(HEADS_PER_CORE):
                        nc.scalar.activation(
                            et[:, h, :nb, :gl], st[:, h, :nb, :gl],
                            mybir.ActivationFunctionType.Exp,
                        )
                else:
                    nc.scalar.activation(
                        et[:, :, :nb, :gl], st[:, :, :nb, :gl],
                        mybir.ActivationFunctionType.Exp,
                    )
                # PVs from two steps back run on PE in this exp's shadow;
                # depth-2 pipelining hides the S->exp->PV semaphore latency
                pending.append((et, step))
                if len(pending) > 2:
                    emit_pv(*pending.popleft())
            while pending:
                emit_pv(*pending.popleft())

            # output DMAs per doc-group on sync; one dma_start covers both
            # heads (combined [65, 2, N] fp16 tensor).  The final doc goes
            # out in two pieces on the scalar ring (idle once exps finish,
            # so its ~0.7us HWDGE descgens overlap the sync ring's work and
            # the last tiny piece chains right after the final CAST).
            for grp in _doc_groups(segs, pieces=(4, 8, 12, 15, 16))[:-1]:
                gs, ge = grp[0][0], grp[-1][1]
                nc.sync.dma_start(ot_d[:, :, gs:ge], ot_sb[:, :, gs:ge])
            ls, le = segs[-1][0], segs[-1][1]
            nc.scalar.dma_start(ot_d[:, :, ls:le], ot_sb[:, :, ls:le])

    nc.compile()
    return nc


def _get_program(segs):
    key = tuple(segs)
    if key not in _prog_cache:
        _prog_cache[key] = _build_program(segs)
    return _prog_cache[key]


def _prep_inputs(Q, K, V, segs):
    """Per-core input maps with host-side layout prep."""
    Q = np.asarray(Q, dtype=np.float32)
    K = np.asarray(K, dtype=np.float32)
    V = np.asarray(V, dtype=np.float32)
    nblk = sum(nb for (_, _, _, nb) in segs)
    # padded index for each real token
    pidx = np.concatenate(
        [128 * b0 + np.arange(e - s) for (s, e, b0, nb) in segs]
    )
    in_maps = []
    for c in range(N_CORES):
        m = {}
        ha = HEADS_PER_CORE * c
        qt = np.concatenate(
            [Q[0, ha + h].T for h in range(HEADS_PER_CORE)], axis=0
        ) * np.float32(SCALE)
        m["qt"] = np.ascontiguousarray(qt.astype(np.float16))
        kt = np.zeros((128, nblk * 128), dtype=np.float16)
        kt[:, pidx] = np.concatenate(
            [K[0, ha + h].T for h in range(HEADS_PER_CORE)], axis=0
        ).astype(np.float16)
        m["kt"] = kt
        for h in range(HEADS_PER_CORE):
            vp = np.zeros((nblk * 128, 65), dtype=np.float16)
            vp[pidx, :64] = V[0, ha + h].astype(np.float16)
            vp[pidx, 64] = 1.0
            m[f"vo{h}"] = np.ascontiguousarray(
                vp.reshape(nblk, 128, 65).transpose(1, 0, 2).reshape(128, nblk * 65)
            )
        in_maps.append(m)
    return in_maps


def _postprocess(results, segs):
    """Normalize + transpose + gather to the full [1, H, N, D] output."""
    out = np.empty((B, H, N, D), dtype=np.float32)
    for c in range(N_CORES):
        ot = np.asarray(results[c]["ot"], dtype=np.float32)  # [65, 2, N]
        for h in range(HEADS_PER_CORE):
            # rows 0-63 numerator, row 64 softmax denominator
            out[0, HEADS_PER_CORE * c + h] = (ot[:64, h] / ot[64:65, h]).T
    return out


def kernel_run(Q, K, V, document_id, trace=False):
    from concourse.bass_utils import run_bass_kernel_spmd

    segs = _doc_segments(document_id)
    nc = _get_program(segs)
    in_maps = _prep_inputs(Q, K, V, segs)
    r = run_bass_kernel_spmd(nc, in_maps, list(range(N_CORES)), trace=trace)
    return _postprocess(r.results, segs), r.exec_time_ns


def kernel(Q, K, V, document_id):
    out, _ = kernel_run(Q, K, V, document_id)
    return out

